# revision 78
# baseline (speedup 1.0000x reference)
"""Multi-head causal self-attention on 8 Trainium2 NeuronCores (Bass/Tile).

Problem: y = proj(softmax(causal_mask(Q K^T / sqrt(D))) V) for B=2, T=2048,
C=1024, H=16 heads, D=64.

Sharding (tensor-parallel over heads, 8-way):
  - Core i owns heads {2i, 2i+1}: computes qT/kT/vT for its heads over both
    batches (full x, its 128-column slice of Wqkv) and runs causal attention
    per head fully on-core, producing normalized yT_local (head-dims on
    partitions, time on the free axis).
  - One 8-way AllToAll per batch reshards head-split -> time-split: core j
    ends with ytf[b] tiles [128, 256] covering time cols [256j, 256j+256)
    of batch b for all heads, and computes out rows = [b0 slice; b1 slice]
    via y @ Wproj.  Host concatenates the 8 col-slices per batch.

Processing is batch-major: attn(b=0) for BOTH heads (their K=64 S^T matmuls
land in different PE row-groups and run concurrently), then the b=0
AllToAll overlaps attn(b=1), and proj(b=0) overlaps the b=1 AllToAll, so
only a 0.5MB collective + half the projection remain on the tail.

All tensors are bf16 (fp32 PSUM accumulation).  Attention is computed
transposed (S^T[k, q], keys on partitions): no transposes in the
attention path, exp on ScalarE straight out of PSUM, and the denominator
comes free from a ones column appended to V (row 64 of the P@V
accumulator).  Causality is exact: S^T blocks strictly above the diagonal
are skipped, diagonal blocks use a restricted column range plus a
triangular multiplicative mask after exp.  Full blocks are computed in
1024-wide pairs so one ACTIVATE covers two blocks.

The kernel is emitted with interleaved instruction streams (generators):
batch-1 projections are woven into batch-0 attention and the b=0 output
projection into batch-1 attention, and each chunk's P@V matmuls are
emitted one chunk late, so the in-order PE queue always has independent
matmuls to run while ScalarE works through exp.  Softmax normalization
runs per q-block (reciprocal on DVE, partition-broadcast on GpSimd,
multiply deferred one block so no engine queue ever waits on the
cross-engine chain); the final block of the last batch uses a fused
short chain (PSUM-evacuation folded into the normalization multiply, one
lane-parallel packed reciprocal, no reshape DMAs) since it alone gates
the last AllToAll trigger.  A tiny barrier AllToAll at kernel start
absorbs cross-core launch skew during the DMA-bound prologue.
"""

import numpy as np
import ml_dtypes

import concourse.bass as bass
import concourse.mybir as mybir
import concourse.tile as tile
from concourse import bacc
from concourse import bass_utils

F32 = mybir.dt.float32
F32R = mybir.dt.float32r
BF16 = mybir.dt.bfloat16
AF = mybir.ActivationFunctionType

B, T, C = 2, 2048, 1024
H, D = 16, 64
N_CORES = 8
HL = H // N_CORES        # heads per core = 2
NCT = C // 128           # contraction tiles = 8
NQ = T // 512            # q tiles per batch = 4
NK = T // 128            # k tiles per batch = 16
SCALE = 1.0 / float(np.sqrt(D))  # 0.125

_BUILD_CACHE = {}


def _drain(*gens, turns=None):
    """Round-robin the generators until all are exhausted.  turns[i] gives
    generator i that many next() calls per round (default 1)."""
    active = list(gens)
    tmap = {id(g): (turns[i] if turns else 1) for i, g in enumerate(gens)}
    while active:
        nxt = []
        for g in active:
            alive = True
            for _ in range(tmap[id(g)]):
                try:
                    next(g)
                except StopIteration:
                    alive = False
                    break
            if alive:
                nxt.append(g)
        active = nxt


def _chain(*gens):
    for g in gens:
        yield from g


def build_kernel(apply_pad_mask: bool):
    nc = bacc.Bacc(
        "TRN2", target_bir_lowering=False, debug=False, num_devices=N_CORES
    )
    xT = nc.dram_tensor("xT", [C, B * T], BF16, kind="ExternalInput").ap()
    wqkv = nc.dram_tensor("wqkv", [C, 3 * HL * D], BF16, kind="ExternalInput").ap()
    wo = nc.dram_tensor("wo", [C, C], BF16, kind="ExternalInput").ap()
    tri = nc.dram_tensor("tri", [128, 128], BF16, kind="ExternalInput").ap()
    ident = nc.dram_tensor("ident", [128, 128], BF16, kind="ExternalInput").ap()
    padk = nc.dram_tensor("padk", [128, B * NK], BF16, kind="ExternalInput").ap()
    out = nc.dram_tensor("out", [512, C], BF16, kind="ExternalOutput").ap()

    with tile.TileContext(nc) as tc:
        with (
            tc.tile_pool(name="const", bufs=1) as constp,
            tc.tile_pool(name="qk", bufs=1) as qkp,
            tc.tile_pool(name="vv", bufs=1) as vvp,
            tc.tile_pool(name="xw", bufs=1) as xwp,
            tc.tile_pool(name="work", bufs=2) as wk,
            tc.tile_pool(name="ytmp_pool", bufs=2) as ytp,
            tc.tile_pool(name="ps_ss", bufs=2, space="PSUM") as ps_ss,
            tc.tile_pool(name="ps_main", bufs=2, space="PSUM") as ps_main,
            tc.tile_pool(name="ps_y", bufs=1, space="PSUM") as ps_y,
            tc.tile_pool(name="dram", bufs=1, space="DRAM") as dram,
        ):
            # -------- critical-path DMAs first: x(b=0) n=0 + wqkv --------
            xt = {b: [xwp.tile([128, NCT, 512], BF16, name=f"xt{b}_{n}",
                               tag=f"xt{b}_{n}") for n in range(NQ)]
                  for b in range(B)}
            for ct in range(NCT):
                nc.sync.dma_start(xt[0][0][:, ct, :],
                                  xT[ct * 128:(ct + 1) * 128, 0:512])
            # weights dispatch from the (idle) Scalar queue so descriptor
            # generation for x and w runs in parallel across two sequencers
            wq_all = xwp.tile([128, NCT, 3 * HL * D], BF16, name="wq_all")
            for ct in range(NCT):
                nc.scalar.dma_start(wq_all[:, ct, :],
                                    wqkv[ct * 128:(ct + 1) * 128, :])

            # ---------------- constants ----------------
            tri_sb = constp.tile([128, 128], BF16, name="tri_sb")
            nc.sync.dma_start(tri_sb[:], tri[:])
            id_sb = constp.tile([128, 128], BF16, name="id_sb")
            nc.sync.dma_start(id_sb[:], ident[:])
            onesc_f = constp.tile([128, HL], F32, name="onesc_f")
            nc.vector.memset(onesc_f[:], 1.0)
            onesc = constp.tile([128, HL], BF16, name="onesc")
            nc.gpsimd.tensor_copy(onesc[:], onesc_f[:])
            # pre-create all V tiles and write their ones columns up front,
            # so GpSimd never interleaves copies into the norm-chain stream
            V = [[None] * NK for _ in range(B)]
            for b in range(B):
                for kt in range(NK):
                    v_sb = vvp.tile([128, HL * 65], BF16, name=f"V{b}_{kt}",
                                    tag=f"V{b}_{kt}")
                    v3 = v_sb[:].rearrange("p (h e) -> p h e", h=HL)
                    nc.gpsimd.tensor_copy(v3[:, :, 64], onesc[:])
                    V[b][kt] = v_sb
            if apply_pad_mask:
                padk_sb = constp.tile([128, B * NK], BF16, name="padk_sb")
                nc.sync.dma_start(padk_sb[:], padk[:])
            # warm the exp table before any real exp lands on ScalarE
            warm = constp.tile([1, 16], F32, name="warm")
            nc.vector.memset(warm[:], 0.0)
            nc.scalar.activation(warm[:], warm[:], AF.Exp)
            # packed denominators for the final fused norm chain: rows at
            # partitions 0 and 32 so ONE lane-parallel reciprocal covers
            # both heads; memset keeps the unused lanes finite
            s2d = constp.tile([33, 512], BF16, name="s2d")
            nc.vector.memset(s2d[:], 1.0)
            r2d = constp.tile([33, 512], BF16, name="r2d")

            a2a_in = [dram.tile([N_CORES, 128, 256], BF16, name=f"a2a_in{b}")
                      for b in range(B)]
            a2a_out = [dram.tile([N_CORES, 128, 256], BF16, name=f"a2a_out{b}")
                       for b in range(B)]
            # tiny barrier collective: absorbs cross-core launch skew during
            # the DMA-bound startup so the real a2a(b=0) sees no peer wait
            bar_in = dram.tile([N_CORES, 1, 16], BF16, name="bar_in")
            bar_out = dram.tile([N_CORES, 1, 16], BF16, name="bar_out")
            barz = constp.tile([1, N_CORES * 16], BF16, name="barz")
            nc.vector.memset(barz[:], 0.0)
            nc.sync.dma_start(
                bar_in[:].rearrange("s p c -> p (s c)"), barz[:])
            nc.gpsimd.collective_compute(
                "AllToAll", mybir.AluOpType.bypass,
                replica_groups=[list(range(N_CORES))],
                ins=[bar_in.opt()], outs=[bar_out.opt()],
            )

            qT = [None] * B
            kT = [None] * B
            ytn = [[None] * (B * NQ) for _ in range(HL)]

            vTs = {}

            def qkv_emit(b, ns):
                """Projections for batch b over q-tiles `ns`, emitted as
                per-n triplets (k, q, v + the v transposes) so attention
                j=n can start as soon as triplet n has landed.  Yields are
                ~1us quanta so interleaved attention chunks are not
                delayed long."""
                for n in ns:
                    if b == 0 and n == 0:
                        continue
                    for ct in range(NCT):
                        nc.sync.dma_start(
                            xt[b][n][:, ct, :],
                            xT[ct * 128:(ct + 1) * 128,
                               b * T + n * 512:b * T + (n + 1) * 512],
                        )
                if qT[b] is None:
                    qT[b] = qkp.tile([128, T], BF16, name="qT", tag=f"qT{b}")
                    kT[b] = qkp.tile([128, T], BF16, name="kT", tag=f"kT{b}")
                    vTs[b] = qkp.tile([128, T], BF16, name="vT",
                                      tag=f"vT{b}")
                vT = vTs[b]
                for n in ns:
                    for which, dst in ((1, kT[b]), (0, qT[b]), (2, vT)):
                        p = ps_main.tile([128, 512], F32, name="p_mm",
                                         tag="ps")
                        for ct in range(NCT):
                            nc.tensor.matmul(
                                p[:],
                                wq_all[:, ct,
                                       which * 128:(which + 1) * 128],
                                xt[b][n][:, ct, :],
                                start=(ct == 0),
                                stop=(ct == NCT - 1),
                            )
                            if ct == 3:
                                yield
                        nc.vector.tensor_copy(dst[:, n * 512:(n + 1) * 512],
                                              p[:])
                        yield
                    for kt in range(4 * n, 4 * n + 4):
                        pt = ps_main.tile([128, 128], BF16, name="p_tr",
                                          tag="ps")
                        nc.tensor.transpose(pt[:],
                                            vT[:, kt * 128:(kt + 1) * 128],
                                            id_sb[:])
                        v3 = V[b][kt][:].rearrange("p (h e) -> p h e", h=HL)
                        nc.vector.tensor_copy(
                            v3[:, :, 0:64],
                            pt[:].rearrange("p (h e) -> p h e", h=HL),
                        )
                        if kt % 2 == 1:
                            yield

            def attn_emit(b):
                """Attention for batch b, both heads interleaved so their
                K=64 S^T matmuls run in different PE row-groups
                concurrently.  Yields per exp-block."""
                coll = [[None] * NQ for _ in range(HL)]
                py = [None] * HL
                norm_pending = [None]
                for j in range(NQ):
                    q0 = j * 512
                    for h in range(HL):
                        py[h] = ps_y.tile([65, 512], F32, name=f"p_y{h}",
                                          tag=f"py{h}")
                    n_kt = 4 * j + 4
                    # paired full blocks, then restricted diagonal singles
                    chunks = []
                    kt = 0
                    while kt < 4 * j:
                        chunks.append((kt, kt + 1))
                        kt += 2
                    for kt in range(4 * j, n_kt):
                        chunks.append((kt,))
                    def make_pv(chunk, p_sbs, j, n_kt, pyl):
                        def emit():
                            for h in range(HL):
                                for ci, kt in enumerate(chunk):
                                    i = kt - 4 * j
                                    off = 128 * i if i >= 0 else 0
                                    base = 512 * ci
                                    nc.tensor.matmul(
                                        pyl[h][0:65, off:512],
                                        V[b][kt][:, h * 65:(h + 1) * 65],
                                        p_sbs[h][:, base + off:base + 512],
                                        start=(kt == 0),
                                        stop=(kt == n_kt - 1),
                                    )
                        return emit

                    pending = None
                    for chunk in chunks:
                        pss = [None] * HL
                        lo = None
                        for h in range(HL):
                            h0 = h * 64
                            pss[h] = ps_ss.tile([128, 1024], F32, name="p_s",
                                                tag="pss")
                            for ci, kt in enumerate(chunk):
                                i = kt - 4 * j
                                off = 128 * i if i >= 0 else 0
                                base = 512 * ci
                                if lo is None:
                                    lo = base + off
                                nc.tensor.matmul(
                                    pss[h][:, base + off:base + 512],
                                    kT[b][h0:h0 + 64,
                                          kt * 128:(kt + 1) * 128],
                                    qT[b][h0:h0 + 64, q0 + off:q0 + 512],
                                    start=True,
                                    stop=True,
                                )
                        hi = 512 * (len(chunk) - 1) + 512
                        p_sbs = [None] * HL
                        for h in range(HL):
                            p_sbs[h] = wk.tile([128, 1024], BF16, name="p_sb",
                                               tag="p_sb", bufs=6)
                            nc.scalar.activation(
                                p_sbs[h][:, lo:hi], pss[h][:, lo:hi], AF.Exp,
                                scale=float(SCALE),
                            )
                            for ci, kt in enumerate(chunk):
                                i = kt - 4 * j
                                off = 128 * i if i >= 0 else 0
                                base = 512 * ci
                                if i >= 0:
                                    nc.vector.tensor_mul(
                                        p_sbs[h][:, base + off:
                                                 base + off + 128],
                                        p_sbs[h][:, base + off:
                                                 base + off + 128],
                                        tri_sb[:],
                                    )
                                if apply_pad_mask:
                                    nc.vector.tensor_scalar_mul(
                                        p_sbs[h][:, base + off:base + 512],
                                        p_sbs[h][:, base + off:base + 512],
                                        padk_sb[:, b * NK + kt:
                                                b * NK + kt + 1],
                                    )
                        # PV of the PREVIOUS chunk: keeps exp + tri-mul off
                        # the in-order PE queue's critical path
                        if pending is not None:
                            pending()
                        pending = make_pv(chunk, p_sbs, j, n_kt, list(py))
                        yield
                    if pending is not None:
                        pending()
                    # evacuate PV accumulators + normalize this j in place:
                    # the whole chain overlaps the next j's compute, so only
                    # j=3's chain precedes the collective trigger
                    m = b * NQ + j
                    # last j of a batch: shortest possible chain to the
                    # collective trigger -- the denominator row is copied
                    # (partition-shifted) to partition 0 and reciprocal runs
                    # 1-lane there, skipping both reshape DMAs; for the
                    # final batch the evacuation also fuses into the
                    # normalization mul
                    last = (b == B - 1 and j == NQ - 1)
                    last_j = last
                    for h in range(HL):
                        yu = ytp.tile([64, 512], BF16, name=f"ytn{h}",
                                      tag=f"ytn{h}_{m}", bufs=1)
                        ytn[h][m] = yu
                        if not last:
                            nc.vector.tensor_copy(yu[:], py[h][0:64, :])
                        if not last_j:
                            srow = wk.tile([65, 512], F32, name=f"srow{h}",
                                           tag=f"srow{h}", bufs=4)
                            nc.vector.tensor_copy(srow[64:65, :],
                                                  py[h][64:65, :])
                            scol = wk.tile([4, 128], F32, name=f"scol{h}",
                                           tag=f"scol{h}", bufs=4)
                            nc.sync.dma_start(scol[:], srow[64:65, :])
                            coll[h][j] = scol
                        else:
                            nc.vector.tensor_copy(s2d[32 * h:32 * h + 1, :],
                                                  py[h][64:65, :])
                    yield
                    pbs = [None] * HL
                    if last_j:
                        # one lane-parallel reciprocal covers both heads;
                        # head 1's row is then shifted to partition 0 so the
                        # broadcast always reads a partition-0 AP (the
                        # base-32 broadcast is broken on hardware)
                        with nc.allow_low_precision(
                                reason="bf16 softmax denom"):
                            nc.vector.reciprocal(r2d[:], s2d[:])
                        r1 = wk.tile([1, 512], BF16, name="r1", tag="r1",
                                     bufs=1)
                        nc.vector.tensor_copy(r1[:], r2d[32:33, :])
                    for h in range(HL):
                        pb = wk.tile([64, 512], BF16, name=f"p_b{h}",
                                     tag=f"pb{h}", bufs=2)
                        if not last_j:
                            rcol = wk.tile([4, 128], BF16, name=f"rcol{h}",
                                           tag=f"rcol{h}", bufs=4)
                            with nc.allow_low_precision(
                                    reason="bf16 softmax denom"):
                                nc.vector.reciprocal(rcol[:], coll[h][j][:])
                            rr = wk.tile([1, 512], BF16, name=f"rrow{h}",
                                         tag=f"rr{h}", bufs=4)
                            nc.sync.dma_start(rr[:], rcol[:])
                            src = rr
                        elif h == 0:
                            src = r2d
                        else:
                            src = r1
                        # broadcast on (idle) GpSimd: its only steady-state op
                        nc.gpsimd.partition_broadcast(pb[:], src[0:1, :])
                        pbs[h] = pb

                    def norm_mul(jj, mm, pbl, fused):
                        def emit():
                            for h in range(HL):
                                if fused is not None:
                                    nc.vector.tensor_mul(ytn[h][mm][:],
                                                         fused[h][0:64, :],
                                                         pbl[h][:])
                                else:
                                    nc.vector.tensor_mul(ytn[h][mm][:],
                                                         ytn[h][mm][:],
                                                         pbl[h][:])
                                # one DMA scatters both 256-col halves into
                                # slots 2j and 2j+1
                                dst = a2a_in[b].rearrange("s p c -> p s c")
                                nc.sync.dma_start(
                                    dst[h * 64:(h + 1) * 64,
                                        2 * jj:2 * jj + 2, :],
                                    ytn[h][mm][:].rearrange(
                                        "p (s c) -> p s c", s=2),
                                )
                        return emit

                    # normalize one j late so the DVE never waits on the
                    # cross-engine reciprocal/broadcast chain
                    if norm_pending[0] is not None:
                        norm_pending[0]()
                    norm_pending[0] = norm_mul(j, m, pbs,
                                               list(py) if last else None)
                    yield
                if norm_pending[0] is not None:
                    norm_pending[0]()

            wo_sb = []
            ytf = [None] * B

            def wo_emit():
                # prefetch Wproj while batch-0 attention runs
                w_sb = xwp.tile([128, NCT, C], BF16, name="wo_all")
                nc.sync.dma_start(w_sb[:],
                                  wo.rearrange("(ct p) c -> p ct c", p=128))
                wo_sb.append(w_sb)
                yield

            def proj_emit(b, delay):
                for _ in range(delay):
                    yield
                # two DMAs pull the 8 slots (slots 0-3 land first so the
                # ct-ordered projection matmuls can start sooner).  b=0 goes
                # via GpSimd (already past the collective wait) so the busy
                # Sync queue never stalls behind the a2a completion; b=1
                # dispatches from two idle HWDGE queues in parallel.
                y_all = xwp.tile([128, NCT, 256], BF16, name=f"ytf{b}")
                src = a2a_out[b].rearrange("s p c -> p s c")
                if b == 0:
                    nc.gpsimd.dma_start(y_all[:, 0:4, :], src[:, 0:4, :])
                    nc.gpsimd.dma_start(y_all[:, 4:8, :], src[:, 4:8, :])
                else:
                    nc.sync.dma_start(y_all[:, 0:4, :], src[:, 0:4, :])
                    nc.scalar.dma_start(y_all[:, 4:8, :], src[:, 4:8, :])
                ytf[b] = y_all
                yield
                for mt in range(2):
                    o_sb = wk.tile([128, C], BF16, name="o_sb", tag="o_sb")
                    for n in range(2):
                        po = ps_main.tile([128, 512], F32, name="p_o",
                                          tag="ps")
                        for ct in range(NCT):
                            nc.tensor.matmul(
                                po[:],
                                y_all[:, ct, mt * 128:(mt + 1) * 128],
                                wo_sb[0][:, ct, n * 512:(n + 1) * 512],
                                start=(ct == 0),
                                stop=(ct == NCT - 1),
                            )
                            if ct == 3:
                                yield
                        nc.vector.tensor_copy(o_sb[:, n * 512:(n + 1) * 512],
                                              po[:])
                        # per-half output DMA: the first half's store
                        # overlaps the second half's matmuls, and the final
                        # transfer gating the drain barrier is halved
                        nc.gpsimd.dma_start(
                            out[b * 256 + mt * 128:b * 256 + (mt + 1) * 128,
                                n * 512:(n + 1) * 512],
                            o_sb[:, n * 512:(n + 1) * 512],
                        )
                        yield

            # ---------------- emission schedule ----------------
            g_qkv0 = qkv_emit(0, range(NQ))
            for _ in range(8):          # k0, q0, v0, transposes 0-3
                next(g_qkv0)
            _drain(attn_emit(0), _chain(g_qkv0, qkv_emit(1, [0]), wo_emit()),
                   turns=[1, 2])
            nc.gpsimd.collective_compute(
                "AllToAll", mybir.AluOpType.bypass,
                replica_groups=[list(range(N_CORES))],
                ins=[a2a_in[0].opt()], outs=[a2a_out[0].opt()],
            )
            # batch-1 k/q/v for n>=1 and the b=0 projection fill the PE
            # while batch-1 attention is exp-bound
            _drain(attn_emit(1),
                   _chain(qkv_emit(1, [1, 2, 3]), proj_emit(0, delay=28)),
                   turns=[1, 2])
            nc.gpsimd.collective_compute(
                "AllToAll", mybir.AluOpType.bypass,
                replica_groups=[list(range(N_CORES))],
                ins=[a2a_in[1].opt()], outs=[a2a_out[1].opt()],
            )
            _drain(proj_emit(1, delay=0))

    nc.compile()
    return nc


def _host_inputs(x, tok_mask, Wqkv, Wproj, apply_pad_mask):
    x = np.ascontiguousarray(np.asarray(x, dtype=np.float32))
    Wqkv = np.ascontiguousarray(np.asarray(Wqkv, dtype=np.float32))
    Wproj = np.ascontiguousarray(np.asarray(Wproj, dtype=np.float32))
    bf = ml_dtypes.bfloat16
    xT = np.concatenate([x[b].T for b in range(B)], axis=1).astype(bf)
    wo_b = Wproj.astype(bf)
    r = np.arange(128)
    tri = (r[None, :] >= r[:, None]).astype(bf)  # keep if col >= row
    ident = np.eye(128, dtype=np.float32).astype(bf)
    if apply_pad_mask:
        padk = np.zeros((128, B * NK), np.float32)
        for b in range(B):
            padk[:, b * NK:(b + 1) * NK] = (
                np.asarray(tok_mask[b]).reshape(NK, 128).T.astype(np.float32)
            )
    else:
        padk = np.ones((128, B * NK), np.float32)
    padk = padk.astype(bf)

    in_maps = []
    for core in range(N_CORES):
        cols = slice(core * HL * D, (core + 1) * HL * D)
        wqkv_c = np.concatenate(
            [Wqkv[:, :C][:, cols], Wqkv[:, C:2 * C][:, cols],
             Wqkv[:, 2 * C:][:, cols]],
            axis=1,
        ).astype(bf)
        in_maps.append(
            {
                "xT": xT,
                "wqkv": wqkv_c,
                "wo": wo_b,
                "tri": tri,
                "ident": ident,
                "padk": padk,
            }
        )
    return in_maps


def kernel(x, tok_mask, Wqkv, Wproj, _run_kwargs=None):
    tok = np.asarray(tok_mask)
    apply_pad_mask = not bool(tok.all())
    key = apply_pad_mask
    if key not in _BUILD_CACHE:
        _BUILD_CACHE[key] = build_kernel(apply_pad_mask)
    nc = _BUILD_CACHE[key]
    in_maps = _host_inputs(x, tok_mask, Wqkv, Wproj, apply_pad_mask)
    kw = dict(_run_kwargs or {})
    res = bass_utils.run_bass_kernel_spmd(
        nc, in_maps, core_ids=list(range(N_CORES)), **kw
    )
    out = np.empty((B, T, C), np.float32)
    for core in range(N_CORES):
        o = np.asarray(res.results[core]["out"], dtype=np.float32)
        for b in range(B):
            out[b, core * 256:(core + 1) * 256, :] = o[b * 256:(b + 1) * 256]
    kernel.last_result = res
    return out


# revision 80
# speedup vs baseline: 1.0743x; 1.0743x over previous
"""Multi-head causal self-attention on 8 Trainium2 NeuronCores (Bass/Tile).

Problem: y = proj(softmax(causal_mask(Q K^T / sqrt(D))) V) for B=2, T=2048,
C=1024, H=16 heads, D=64.

Sharding (tensor-parallel over heads, 8-way):
  - Core i owns heads {2i, 2i+1}: computes qT/kT/vT for its heads over both
    batches (full x, its 128-column slice of Wqkv) and runs causal attention
    per head fully on-core, producing normalized yT_local (head-dims on
    partitions, time on the free axis).
  - One 8-way AllToAll per batch reshards head-split -> time-split: core j
    ends with ytf[b] tiles [128, 256] covering time cols [256j, 256j+256)
    of batch b for all heads, and computes out rows = [b0 slice; b1 slice]
    via y @ Wproj.  Host concatenates the 8 col-slices per batch.

Processing is batch-major: attn(b=0) for BOTH heads (their K=64 S^T matmuls
land in different PE row-groups and run concurrently), then the b=0
AllToAll overlaps attn(b=1), and proj(b=0) overlaps the b=1 AllToAll, so
only a 0.5MB collective + half the projection remain on the tail.

All tensors are bf16 (fp32 PSUM accumulation).  Attention is computed
transposed (S^T[k, q], keys on partitions): no transposes in the
attention path, exp on ScalarE straight out of PSUM, and the denominator
comes free from a ones column appended to V (row 64 of the P@V
accumulator).  Causality is exact: S^T blocks strictly above the diagonal
are skipped, diagonal blocks use a restricted column range plus a
triangular multiplicative mask after exp.  Full blocks are computed in
1024-wide pairs so one ACTIVATE covers two blocks.

The kernel is emitted with interleaved instruction streams (generators):
batch-1 projections are woven into batch-0 attention and the b=0 output
projection into batch-1 attention, and each chunk's P@V matmuls are
emitted one chunk late, so the in-order PE queue always has independent
matmuls to run while ScalarE works through exp.  Softmax normalization
runs per q-block (reciprocal on DVE, partition-broadcast on GpSimd,
multiply deferred one block so no engine queue ever waits on the
cross-engine chain); the final block of the last batch uses a fused
short chain (PSUM-evacuation folded into the normalization multiply, one
lane-parallel packed reciprocal, no reshape DMAs) since it alone gates
the last AllToAll trigger.  A tiny barrier AllToAll at kernel start
absorbs cross-core launch skew during the DMA-bound prologue.
"""

import numpy as np
import ml_dtypes

import concourse.bass as bass
import concourse.mybir as mybir
import concourse.tile as tile
from concourse import bacc
from concourse import bass_utils

F32 = mybir.dt.float32
F32R = mybir.dt.float32r
BF16 = mybir.dt.bfloat16
AF = mybir.ActivationFunctionType

B, T, C = 2, 2048, 1024
H, D = 16, 64
N_CORES = 8
HL = H // N_CORES        # heads per core = 2
NCT = C // 128           # contraction tiles = 8
NQ = T // 512            # q tiles per batch = 4
NK = T // 128            # k tiles per batch = 16
SCALE = 1.0 / float(np.sqrt(D))  # 0.125

_BUILD_CACHE = {}


def _drain(*gens, turns=None):
    """Round-robin the generators until all are exhausted.  turns[i] gives
    generator i that many next() calls per round (default 1)."""
    active = list(gens)
    tmap = {id(g): (turns[i] if turns else 1) for i, g in enumerate(gens)}
    while active:
        nxt = []
        for g in active:
            alive = True
            for _ in range(tmap[id(g)]):
                try:
                    next(g)
                except StopIteration:
                    alive = False
                    break
            if alive:
                nxt.append(g)
        active = nxt


def _chain(*gens):
    for g in gens:
        yield from g


def build_kernel(apply_pad_mask: bool):
    nc = bacc.Bacc(
        "TRN2", target_bir_lowering=False, debug=False, num_devices=N_CORES
    )
    xT = nc.dram_tensor("xT", [C, B * T], BF16, kind="ExternalInput").ap()
    wqkv = nc.dram_tensor("wqkv", [C, 3 * HL * D], BF16, kind="ExternalInput").ap()
    wo = nc.dram_tensor("wo", [C, C], BF16, kind="ExternalInput").ap()
    tri = nc.dram_tensor("tri", [128, 128], BF16, kind="ExternalInput").ap()
    ident = nc.dram_tensor("ident", [128, 128], BF16, kind="ExternalInput").ap()
    padk = nc.dram_tensor("padk", [128, B * NK], BF16, kind="ExternalInput").ap()
    out = nc.dram_tensor("out", [512, C], BF16, kind="ExternalOutput").ap()

    with tile.TileContext(nc) as tc:
        with (
            tc.tile_pool(name="const", bufs=1) as constp,
            tc.tile_pool(name="qk", bufs=1) as qkp,
            tc.tile_pool(name="vv", bufs=1) as vvp,
            tc.tile_pool(name="xw", bufs=1) as xwp,
            tc.tile_pool(name="work", bufs=2) as wk,
            tc.tile_pool(name="ytmp_pool", bufs=2) as ytp,
            tc.tile_pool(name="ps_ss", bufs=2, space="PSUM") as ps_ss,
            tc.tile_pool(name="ps_main", bufs=2, space="PSUM") as ps_main,
            tc.tile_pool(name="ps_y", bufs=1, space="PSUM") as ps_y,
            tc.tile_pool(name="dram", bufs=1, space="DRAM") as dram,
        ):
            # -------- critical-path DMAs first: x(b=0) n=0 + wqkv --------
            xt = {b: [xwp.tile([128, NCT, 512], BF16, name=f"xt{b}_{n}",
                               tag=f"xt{b}_{n}") for n in range(NQ)]
                  for b in range(B)}
            for ct in range(NCT):
                nc.sync.dma_start(xt[0][0][:, ct, :],
                                  xT[ct * 128:(ct + 1) * 128, 0:512])
            # weights dispatch from the (idle) Scalar queue so descriptor
            # generation for x and w runs in parallel across two sequencers
            wq_all = xwp.tile([128, NCT, 3 * HL * D], BF16, name="wq_all")
            for ct in range(NCT):
                nc.scalar.dma_start(wq_all[:, ct, :],
                                    wqkv[ct * 128:(ct + 1) * 128, :])

            # ---------------- constants ----------------
            tri_sb = constp.tile([128, 128], BF16, name="tri_sb")
            nc.sync.dma_start(tri_sb[:], tri[:])
            id_sb = constp.tile([128, 128], BF16, name="id_sb")
            nc.sync.dma_start(id_sb[:], ident[:])
            onesc_f = constp.tile([128, HL], F32, name="onesc_f")
            nc.vector.memset(onesc_f[:], 1.0)
            onesc = constp.tile([128, HL], BF16, name="onesc")
            nc.gpsimd.tensor_copy(onesc[:], onesc_f[:])
            # pre-create all V tiles and write their ones columns up front,
            # so GpSimd never interleaves copies into the norm-chain stream
            V = [[None] * NK for _ in range(B)]
            for b in range(B):
                for kt in range(NK):
                    v_sb = vvp.tile([128, HL * 65], BF16, name=f"V{b}_{kt}",
                                    tag=f"V{b}_{kt}")
                    v3 = v_sb[:].rearrange("p (h e) -> p h e", h=HL)
                    nc.gpsimd.tensor_copy(v3[:, :, 64], onesc[:])
                    V[b][kt] = v_sb
            if apply_pad_mask:
                padk_sb = constp.tile([128, B * NK], BF16, name="padk_sb")
                nc.sync.dma_start(padk_sb[:], padk[:])
            # warm the exp table before any real exp lands on ScalarE
            warm = constp.tile([1, 16], F32, name="warm")
            nc.vector.memset(warm[:], 0.0)
            nc.scalar.activation(warm[:], warm[:], AF.Exp)
            # packed denominators for the final fused norm chain: rows at
            # partitions 0 and 32 so ONE lane-parallel reciprocal covers
            # both heads; memset keeps the unused lanes finite
            s2d = constp.tile([33, 512], BF16, name="s2d")
            nc.vector.memset(s2d[:], 1.0)
            r2d = constp.tile([33, 512], BF16, name="r2d")

            a2a_in = [dram.tile([N_CORES, 128, 256], BF16, name=f"a2a_in{b}")
                      for b in range(B)]
            a2a_out = [dram.tile([N_CORES, 128, 256], BF16, name=f"a2a_out{b}")
                       for b in range(B)]
            # tiny barrier collective: absorbs cross-core launch skew during
            # the DMA-bound startup so the real a2a(b=0) sees no peer wait
            bar_in = dram.tile([N_CORES, 1, 16], BF16, name="bar_in")
            bar_out = dram.tile([N_CORES, 1, 16], BF16, name="bar_out")
            barz = constp.tile([1, N_CORES * 16], BF16, name="barz")
            nc.vector.memset(barz[:], 0.0)
            nc.sync.dma_start(
                bar_in[:].rearrange("s p c -> p (s c)"), barz[:])
            nc.gpsimd.collective_compute(
                "AllToAll", mybir.AluOpType.bypass,
                replica_groups=[list(range(N_CORES))],
                ins=[bar_in.opt()], outs=[bar_out.opt()],
            )

            qT = [None] * B
            kT = [None] * B
            ytn = [[None] * (B * NQ) for _ in range(HL)]

            vTs = {}

            def qkv_emit(b, ns):
                """Projections for batch b over q-tiles `ns`, emitted as
                per-n triplets (k, q, v + the v transposes) so attention
                j=n can start as soon as triplet n has landed.  Yields are
                ~1us quanta so interleaved attention chunks are not
                delayed long."""
                for n in ns:
                    if b == 0 and n == 0:
                        continue
                    for ct in range(NCT):
                        nc.sync.dma_start(
                            xt[b][n][:, ct, :],
                            xT[ct * 128:(ct + 1) * 128,
                               b * T + n * 512:b * T + (n + 1) * 512],
                        )
                if qT[b] is None:
                    qT[b] = qkp.tile([128, T], BF16, name="qT", tag=f"qT{b}")
                    kT[b] = qkp.tile([128, T], BF16, name="kT", tag=f"kT{b}")
                    vTs[b] = qkp.tile([128, T], BF16, name="vT",
                                      tag=f"vT{b}")
                vT = vTs[b]
                for n in ns:
                    for which, dst in ((1, kT[b]), (0, qT[b]), (2, vT)):
                        p = ps_main.tile([128, 512], F32, name="p_mm",
                                         tag="ps")
                        for ct in range(NCT):
                            nc.tensor.matmul(
                                p[:],
                                wq_all[:, ct,
                                       which * 128:(which + 1) * 128],
                                xt[b][n][:, ct, :],
                                start=(ct == 0),
                                stop=(ct == NCT - 1),
                            )
                            if ct == 3:
                                yield
                        nc.vector.tensor_copy(dst[:, n * 512:(n + 1) * 512],
                                              p[:])
                        yield
                    for kt in range(4 * n, 4 * n + 4):
                        pt = ps_main.tile([128, 128], BF16, name="p_tr",
                                          tag="ps")
                        nc.tensor.transpose(pt[:],
                                            vT[:, kt * 128:(kt + 1) * 128],
                                            id_sb[:])
                        v3 = V[b][kt][:].rearrange("p (h e) -> p h e", h=HL)
                        nc.vector.tensor_copy(
                            v3[:, :, 0:64],
                            pt[:].rearrange("p (h e) -> p h e", h=HL),
                        )
                        if kt % 2 == 1:
                            yield

            def attn_emit(b):
                """Attention for batch b, both heads interleaved so their
                K=64 S^T matmuls run in different PE row-groups
                concurrently.  Yields per exp-block."""
                coll = [[None] * NQ for _ in range(HL)]
                py = [None] * HL
                norm_pending = [None]
                for j in range(NQ):
                    q0 = j * 512
                    for h in range(HL):
                        py[h] = ps_y.tile([65, 512], F32, name=f"p_y{h}",
                                          tag=f"py{h}")
                    n_kt = 4 * j + 4
                    # paired full blocks, then restricted diagonal singles
                    chunks = []
                    kt = 0
                    while kt < 4 * j:
                        chunks.append((kt, kt + 1))
                        kt += 2
                    for kt in range(4 * j, n_kt):
                        chunks.append((kt,))
                    def make_pv(chunk, p_sbs, j, n_kt, pyl):
                        def emit():
                            for h in range(HL):
                                for ci, kt in enumerate(chunk):
                                    i = kt - 4 * j
                                    off = 128 * i if i >= 0 else 0
                                    base = 512 * ci
                                    nc.tensor.matmul(
                                        pyl[h][0:65, off:512],
                                        V[b][kt][:, h * 65:(h + 1) * 65],
                                        p_sbs[h][:, base + off:base + 512],
                                        start=(kt == 0),
                                        stop=(kt == n_kt - 1),
                                    )
                        return emit

                    pending = None
                    for chunk in chunks:
                        pss = [None] * HL
                        lo = None
                        for h in range(HL):
                            h0 = h * 64
                            pss[h] = ps_ss.tile([128, 1024], F32, name="p_s",
                                                tag="pss")
                            for ci, kt in enumerate(chunk):
                                i = kt - 4 * j
                                off = 128 * i if i >= 0 else 0
                                base = 512 * ci
                                if lo is None:
                                    lo = base + off
                                nc.tensor.matmul(
                                    pss[h][:, base + off:base + 512],
                                    kT[b][h0:h0 + 64,
                                          kt * 128:(kt + 1) * 128],
                                    qT[b][h0:h0 + 64, q0 + off:q0 + 512],
                                    start=True,
                                    stop=True,
                                )
                        hi = 512 * (len(chunk) - 1) + 512
                        p_sbs = [None] * HL
                        for h in range(HL):
                            p_sbs[h] = wk.tile([128, 1024], BF16, name="p_sb",
                                               tag="p_sb", bufs=6)
                            nc.scalar.activation(
                                p_sbs[h][:, lo:hi], pss[h][:, lo:hi], AF.Exp,
                                scale=float(SCALE),
                            )
                            for ci, kt in enumerate(chunk):
                                i = kt - 4 * j
                                off = 128 * i if i >= 0 else 0
                                base = 512 * ci
                                if i >= 0:
                                    nc.vector.tensor_mul(
                                        p_sbs[h][:, base + off:
                                                 base + off + 128],
                                        p_sbs[h][:, base + off:
                                                 base + off + 128],
                                        tri_sb[:],
                                    )
                                if apply_pad_mask:
                                    nc.vector.tensor_scalar_mul(
                                        p_sbs[h][:, base + off:base + 512],
                                        p_sbs[h][:, base + off:base + 512],
                                        padk_sb[:, b * NK + kt:
                                                b * NK + kt + 1],
                                    )
                        # PV of the PREVIOUS chunk: keeps exp + tri-mul off
                        # the in-order PE queue's critical path
                        if pending is not None:
                            pending()
                        pending = make_pv(chunk, p_sbs, j, n_kt, list(py))
                        yield
                    if pending is not None:
                        pending()
                    # evacuate PV accumulators + normalize this j in place:
                    # the whole chain overlaps the next j's compute, so only
                    # j=3's chain precedes the collective trigger
                    m = b * NQ + j
                    # last j of a batch: shortest possible chain to the
                    # collective trigger -- the denominator row is copied
                    # (partition-shifted) to partition 0 and reciprocal runs
                    # 1-lane there, skipping both reshape DMAs; for the
                    # final batch the evacuation also fuses into the
                    # normalization mul
                    last = (b == B - 1 and j == NQ - 1)
                    last_j = last
                    for h in range(HL):
                        yu = ytp.tile([64, 512], BF16, name=f"ytn{h}",
                                      tag=f"ytn{h}_{m}", bufs=1)
                        ytn[h][m] = yu
                        if not last:
                            nc.vector.tensor_copy(yu[:], py[h][0:64, :])
                        if not last_j:
                            if h == 0:
                                sc2 = wk.tile([8, 128], F32, name="sc2",
                                              tag="sc2", bufs=4)
                            srow = wk.tile([65, 512], F32, name=f"srow{h}",
                                           tag=f"srow{h}", bufs=4)
                            nc.vector.tensor_copy(srow[64:65, :],
                                                  py[h][64:65, :])
                            nc.sync.dma_start(sc2[4 * h:4 * h + 4, :],
                                              srow[64:65, :])
                            coll[h][j] = sc2
                        else:
                            nc.vector.tensor_copy(s2d[32 * h:32 * h + 1, :],
                                                  py[h][64:65, :])
                    yield
                    pbs = [None] * HL
                    if last_j:
                        # one lane-parallel reciprocal covers both heads;
                        # head 1's row is then shifted to partition 0 so the
                        # broadcast always reads a partition-0 AP (the
                        # base-32 broadcast is broken on hardware)
                        with nc.allow_low_precision(
                                reason="bf16 softmax denom"):
                            nc.vector.reciprocal(r2d[:], s2d[:])
                        r1 = wk.tile([1, 512], BF16, name="r1", tag="r1",
                                     bufs=1)
                        nc.vector.tensor_copy(r1[:], r2d[32:33, :])
                    if not last_j:
                        # one reciprocal covers both heads' packed rows:
                        # halves the DVE occupancy at every j boundary
                        rc2 = wk.tile([8, 128], BF16, name="rc2",
                                      tag="rc2", bufs=4)
                        with nc.allow_low_precision(
                                reason="bf16 softmax denom"):
                            nc.vector.reciprocal(rc2[:], coll[0][j][:])
                    for h in range(HL):
                        pb = wk.tile([64, 512], BF16, name=f"p_b{h}",
                                     tag=f"pb{h}", bufs=2)
                        if not last_j:
                            rr = wk.tile([1, 512], BF16, name=f"rrow{h}",
                                         tag=f"rr{h}", bufs=4)
                            nc.sync.dma_start(rr[:], rc2[4 * h:4 * h + 4, :])
                            src = rr
                        elif h == 0:
                            src = r2d
                        else:
                            src = r1
                        # broadcast on (idle) GpSimd: its only steady-state op
                        nc.gpsimd.partition_broadcast(pb[:], src[0:1, :])
                        pbs[h] = pb

                    def norm_mul(jj, mm, pbl, fused):
                        def emit():
                            for h in range(HL):
                                if fused is not None:
                                    nc.vector.tensor_mul(ytn[h][mm][:],
                                                         fused[h][0:64, :],
                                                         pbl[h][:])
                                else:
                                    nc.vector.tensor_mul(ytn[h][mm][:],
                                                         ytn[h][mm][:],
                                                         pbl[h][:])
                                # one DMA scatters both 256-col halves into
                                # slots 2j and 2j+1
                                dst = a2a_in[b].rearrange("s p c -> p s c")
                                nc.sync.dma_start(
                                    dst[h * 64:(h + 1) * 64,
                                        2 * jj:2 * jj + 2, :],
                                    ytn[h][mm][:].rearrange(
                                        "p (s c) -> p s c", s=2),
                                )
                        return emit

                    # normalize one j late so the DVE never waits on the
                    # cross-engine reciprocal/broadcast chain
                    if norm_pending[0] is not None:
                        norm_pending[0]()
                    norm_pending[0] = norm_mul(j, m, pbs,
                                               list(py) if last else None)
                    yield
                if norm_pending[0] is not None:
                    norm_pending[0]()

            wo_sb = []
            ytf = [None] * B

            def wo_emit():
                # prefetch Wproj while batch-0 attention runs
                w_sb = xwp.tile([128, NCT, C], BF16, name="wo_all")
                nc.sync.dma_start(w_sb[:],
                                  wo.rearrange("(ct p) c -> p ct c", p=128))
                wo_sb.append(w_sb)
                yield

            def proj_emit(b, delay):
                for _ in range(delay):
                    yield
                # two DMAs pull the 8 slots (slots 0-3 land first so the
                # ct-ordered projection matmuls can start sooner).  b=0 goes
                # via GpSimd (already past the collective wait) so the busy
                # Sync queue never stalls behind the a2a completion; b=1
                # dispatches from two idle HWDGE queues in parallel.
                y_all = xwp.tile([128, NCT, 256], BF16, name=f"ytf{b}")
                src = a2a_out[b].rearrange("s p c -> p s c")
                if b == 0:
                    nc.gpsimd.dma_start(y_all[:, 0:4, :], src[:, 0:4, :])
                    nc.gpsimd.dma_start(y_all[:, 4:8, :], src[:, 4:8, :])
                else:
                    nc.sync.dma_start(y_all[:, 0:4, :], src[:, 0:4, :])
                    nc.scalar.dma_start(y_all[:, 4:8, :], src[:, 4:8, :])
                ytf[b] = y_all
                yield
                for mt in range(2):
                    o_sb = wk.tile([128, C], BF16, name="o_sb", tag="o_sb")
                    for n in range(2):
                        po = ps_main.tile([128, 512], F32, name="p_o",
                                          tag="ps")
                        for ct in range(NCT):
                            nc.tensor.matmul(
                                po[:],
                                y_all[:, ct, mt * 128:(mt + 1) * 128],
                                wo_sb[0][:, ct, n * 512:(n + 1) * 512],
                                start=(ct == 0),
                                stop=(ct == NCT - 1),
                            )
                            if ct == 3:
                                yield
                        nc.vector.tensor_copy(o_sb[:, n * 512:(n + 1) * 512],
                                              po[:])
                        # per-half output DMA: the first half's store
                        # overlaps the second half's matmuls, and the final
                        # transfer gating the drain barrier is halved
                        nc.gpsimd.dma_start(
                            out[b * 256 + mt * 128:b * 256 + (mt + 1) * 128,
                                n * 512:(n + 1) * 512],
                            o_sb[:, n * 512:(n + 1) * 512],
                        )
                        yield

            # ---------------- emission schedule ----------------
            g_qkv0 = qkv_emit(0, range(NQ))
            for _ in range(8):          # k0, q0, v0, transposes 0-3
                next(g_qkv0)
            _drain(attn_emit(0), _chain(g_qkv0, qkv_emit(1, [0]), wo_emit()),
                   turns=[1, 2])
            nc.gpsimd.collective_compute(
                "AllToAll", mybir.AluOpType.bypass,
                replica_groups=[list(range(N_CORES))],
                ins=[a2a_in[0].opt()], outs=[a2a_out[0].opt()],
            )
            # batch-1 k/q/v for n>=1 and the b=0 projection fill the PE
            # while batch-1 attention is exp-bound
            _drain(attn_emit(1),
                   _chain(qkv_emit(1, [1, 2, 3]), proj_emit(0, delay=28)),
                   turns=[1, 2])
            nc.gpsimd.collective_compute(
                "AllToAll", mybir.AluOpType.bypass,
                replica_groups=[list(range(N_CORES))],
                ins=[a2a_in[1].opt()], outs=[a2a_out[1].opt()],
            )
            _drain(proj_emit(1, delay=0))

    nc.compile()
    return nc


def _host_inputs(x, tok_mask, Wqkv, Wproj, apply_pad_mask):
    x = np.ascontiguousarray(np.asarray(x, dtype=np.float32))
    Wqkv = np.ascontiguousarray(np.asarray(Wqkv, dtype=np.float32))
    Wproj = np.ascontiguousarray(np.asarray(Wproj, dtype=np.float32))
    bf = ml_dtypes.bfloat16
    xT = np.concatenate([x[b].T for b in range(B)], axis=1).astype(bf)
    wo_b = Wproj.astype(bf)
    r = np.arange(128)
    tri = (r[None, :] >= r[:, None]).astype(bf)  # keep if col >= row
    ident = np.eye(128, dtype=np.float32).astype(bf)
    if apply_pad_mask:
        padk = np.zeros((128, B * NK), np.float32)
        for b in range(B):
            padk[:, b * NK:(b + 1) * NK] = (
                np.asarray(tok_mask[b]).reshape(NK, 128).T.astype(np.float32)
            )
    else:
        padk = np.ones((128, B * NK), np.float32)
    padk = padk.astype(bf)

    in_maps = []
    for core in range(N_CORES):
        cols = slice(core * HL * D, (core + 1) * HL * D)
        wqkv_c = np.concatenate(
            [Wqkv[:, :C][:, cols], Wqkv[:, C:2 * C][:, cols],
             Wqkv[:, 2 * C:][:, cols]],
            axis=1,
        ).astype(bf)
        in_maps.append(
            {
                "xT": xT,
                "wqkv": wqkv_c,
                "wo": wo_b,
                "tri": tri,
                "ident": ident,
                "padk": padk,
            }
        )
    return in_maps


def kernel(x, tok_mask, Wqkv, Wproj, _run_kwargs=None):
    tok = np.asarray(tok_mask)
    apply_pad_mask = not bool(tok.all())
    key = apply_pad_mask
    if key not in _BUILD_CACHE:
        _BUILD_CACHE[key] = build_kernel(apply_pad_mask)
    nc = _BUILD_CACHE[key]
    in_maps = _host_inputs(x, tok_mask, Wqkv, Wproj, apply_pad_mask)
    kw = dict(_run_kwargs or {})
    res = bass_utils.run_bass_kernel_spmd(
        nc, in_maps, core_ids=list(range(N_CORES)), **kw
    )
    out = np.empty((B, T, C), np.float32)
    for core in range(N_CORES):
        o = np.asarray(res.results[core]["out"], dtype=np.float32)
        for b in range(B):
            out[b, core * 256:(core + 1) * 256, :] = o[b * 256:(b + 1) * 256]
    kernel.last_result = res
    return out
